# revision 4
# baseline (speedup 1.0000x reference)
"""Trainium2 Bass kernel for nn_ChargeEq: per-molecule Ewald matrix build +
constrained charge-equilibration solve, data-parallel over 8 NeuronCores.

Each core handles one molecule:
  - builds the reciprocal-space Ewald matrix A [256,256] on-device
    (half k-space + weights folded in, bf16-triple-split phase matmul,
    range-reduced ACT sin)
  - inverts M = A + diag(J) via quadratic Newton-Schulz iteration
    (float32r fast matmuls for growth rounds, fp32 cleanup), refines the
    two solves iteratively, applies the total-charge constraint
  - computes q and the Ewald energy e.
"""
import numpy as np

import concourse.bass as bass
import concourse.mybir as mybir
from concourse import bacc
from concourse.tile import TileContext
from concourse.bass_utils import run_bass_kernel_spmd

F32 = mybir.dt.float32
F32R = mybir.dt.float32r
BF16 = mybir.dt.bfloat16
AFT = mybir.ActivationFunctionType
ALU = mybir.AluOpType

# ---- nn.Module constants ----
B, N = 8, 256
DL, SIGMA = 1.5, 1.0
ELEMENTS = np.array([1, 6, 7, 8])
NORM_FACTOR = (1.0 / 90.0474) ** 0.5
TWO_PI = 2.0 * np.pi
NMAX = 10
GAMMA = 2.0 / (SIGMA * np.sqrt(TWO_PI))

KPAD = 2176            # padded half-k-space count (17 * 128)
KBLK = KPAD // 128
MAGIC = 12582912.0     # 1.5 * 2**23 : fp32 round-to-nearest-int trick

NS_ROUNDS = [(3.5, F32R)] * 16 + [(2.0, F32R)] + [(2.0, F32)] * 2
N_REFINE = 5

_CACHE = {}


def _half_k_grid():
    g = np.arange(-NMAX, NMAX + 1)
    G = np.stack(np.meshgrid(g, g, g, indexing="ij"), -1).reshape(-1, 3)
    G = G.astype(np.float64)
    keep = (G[:, 0] > 0) | ((G[:, 0] == 0) & (G[:, 1] > 0)) | \
           ((G[:, 0] == 0) & (G[:, 1] == 0) & (G[:, 2] > 0))
    return G[keep]


def _bf16(x):
    import ml_dtypes
    return np.asarray(x, np.float32).astype(ml_dtypes.bfloat16)


def _build_program():
    nc = bacc.Bacc(target_bir_lowering=False)

    di = {}
    def dram_in(name, shape, dtype):
        di[name] = nc.dram_tensor(name, shape, dtype, kind="ExternalInput")

    dram_in("kmat", [10, KPAD], BF16)
    dram_in("rK", [10, 512], BF16)
    dram_in("negI", [128, 128], BF16)
    dram_in("posI", [128, 128], F32)
    dram_in("wcol", [128, KBLK], F32)
    dram_in("D0", [128, 256], F32)
    dram_in("D1", [128, 128], F32)
    dram_in("IA", [128, 384], F32)
    dram_in("I2", [128, 384], F32)
    dram_in("bc", [128, 4], F32)
    dram_in("Jq", [128, 2], F32)
    dram_in("cst", [1, 1], F32)
    dram_in("ones", [128, 1], F32)
    dram_in("onesr", [1, 128], F32)

    qout = nc.dram_tensor("qout", [256, 1], F32, kind="ExternalOutput")
    eout = nc.dram_tensor("eout", [1, 1], F32, kind="ExternalOutput")

    def cast(ap, dt):
        return ap if ap.dtype == dt else ap.bitcast(dt)

    with TileContext(nc) as tc:
        with tc.tile_pool(name="cpool", bufs=1) as cpool, \
             tc.tile_pool(name="wpool", bufs=2) as wpool, \
             tc.tile_pool(name="cswp", bufs=3) as cswp, \
             tc.tile_pool(name="mpool", bufs=1) as mpool, \
             tc.tile_pool(name="nsp", bufs=2) as nsp:

            def load(name, shape, dtype):
                t = cpool.tile(shape, dtype, name=f"sb_{name}")
                nc.gpsimd.dma_start(
                    out=t[tuple(slice(0, s) for s in shape)],
                    in_=di[name].ap())
                return t

            kmat_sb = load("kmat", [10, KPAD], BF16)
            rK_sb = load("rK", [10, 512], BF16)
            negI_sb = load("negI", [128, 128], BF16)
            posI_sb = load("posI", [128, 128], F32)
            wcol_sb = load("wcol", [128, KBLK], F32)
            D0_sb = load("D0", [128, 256], F32)
            D1_sb = load("D1", [128, 128], F32)
            IA_sb = load("IA", [128, 384], F32)
            I2_sb = load("I2", [128, 384], F32)
            bc_sb = load("bc", [128, 4], F32)
            Jq_sb = load("Jq", [128, 2], F32)
            cst_sb = load("cst", [1, 1], F32)
            ones_sb = load("ones", [128, 1], F32)
            onesr_sb = load("onesr", [1, 128], F32)

            # ================= Phase A: build A, assemble M =================
            with tc.tile_pool(name="psA", bufs=2, space="PSUM") as psA, \
                 tc.tile_pool(name="psAcc", bufs=1, space="PSUM") as psAcc:
                A0_ps = psAcc.tile([128, 256], F32)
                A1_ps = psAcc.tile([128, 128], F32)
                for b in range(KBLK):
                    t_ps = psA.tile([128, 512], F32, tag="tps")
                    nc.tensor.matmul(t_ps[:, :],
                                     kmat_sb[:, b * 128:(b + 1) * 128],
                                     rK_sb[:, :], start=True, stop=False)
                    a1 = wpool.tile([128, 512], F32, tag="a1")
                    nc.vector.tensor_scalar_add(a1[:, :], t_ps[:, :], MAGIC)
                    rintb = wpool.tile([128, 512], BF16, tag="rintb")
                    nc.vector.tensor_scalar_sub(rintb[:, :], a1[:, :], MAGIC)
                    nc.tensor.matmul(t_ps[:, :], negI_sb[:, :], rintb[:, :],
                                     start=False, stop=True)
                    snc = wpool.tile([128, 512], F32, tag="snc")
                    nc.scalar.activation(snc[:, :], t_ps[:, :], AFT.Sin,
                                         scale=float(TWO_PI))
                    csw = cswp.tile([128, 512], F32, tag="csw")
                    nc.vector.tensor_scalar_mul(csw[:, :], snc[:, :],
                                                wcol_sb[:, b:b + 1])
                    nc.tensor.matmul(A0_ps[:, :], csw[:, 0:128], csw[:, 0:256],
                                     start=(b == 0), stop=False)
                    nc.tensor.matmul(A0_ps[:, :], csw[:, 256:384],
                                     csw[:, 256:512],
                                     start=False, stop=(b == KBLK - 1))
                    nc.tensor.matmul(A1_ps[:, :], csw[:, 128:256],
                                     csw[:, 128:256],
                                     start=(b == 0), stop=False)
                    nc.tensor.matmul(A1_ps[:, :], csw[:, 384:512],
                                     csw[:, 384:512],
                                     start=False, stop=(b == KBLK - 1))

                Mf0 = mpool.tile([128, 256], F32)
                nc.vector.scalar_tensor_tensor(Mf0[:, :], A0_ps[:, :], 1.0,
                                               D0_sb[:, :], ALU.mult, ALU.add)
                tr_ps = psA.tile([128, 128], F32, tag="trps", bufs=1)
                nc.tensor.transpose(tr_ps[:, :], Mf0[:, 128:256], posI_sb[:, :])
                Mf1 = mpool.tile([128, 256], F32)
                nc.vector.tensor_copy(Mf1[:, 0:128], tr_ps[:, :])
                nc.vector.scalar_tensor_tensor(Mf1[:, 128:256], A1_ps[:, :], 1.0,
                                               D1_sb[:, :], ALU.mult, ALU.add)
                Mr0 = mpool.tile([128, 256], F32R)
                nc.vector.tensor_copy(Mr0[:, :], Mf0[:, :])
                Mr1 = mpool.tile([128, 256], F32R)
                nc.vector.tensor_copy(Mr1[:, :], Mf1[:, :])

                # ---- alpha = 1/||M||_1^2, X0 = alpha*M ----
                rs0 = wpool.tile([128, 1], F32, tag="rs0")
                nc.vector.tensor_reduce(rs0[:, :], Mf0[:, :],
                                        axis=mybir.AxisListType.X, op=ALU.add,
                                        apply_absolute_value=True)
                rs1 = wpool.tile([128, 1], F32, tag="rs1")
                nc.vector.tensor_reduce(rs1[:, :], Mf1[:, :],
                                        axis=mybir.AxisListType.X, op=ALU.add,
                                        apply_absolute_value=True)
                rsmax = wpool.tile([128, 1], F32, tag="rsmax")
                nc.vector.tensor_tensor(rsmax[:, :], rs0[:, :], rs1[:, :],
                                        op=ALU.max)
                rst_ps = psA.tile([1, 128], F32, tag="rstps", bufs=1)
                nc.tensor.transpose(rst_ps[:, :], rsmax[:, :], posI_sb[:, :])
                nrm = wpool.tile([1, 1], F32, tag="nrm")
                nc.vector.tensor_reduce(nrm[:, :], rst_ps[:, :],
                                        axis=mybir.AxisListType.X, op=ALU.max)
                nrm2 = wpool.tile([1, 1], F32, tag="nrm2")
                nc.vector.tensor_tensor(nrm2[:, :], nrm[:, :], nrm[:, :],
                                        op=ALU.mult)
                alph = wpool.tile([1, 1], F32, tag="alph")
                nc.vector.reciprocal(alph[:, :], nrm2[:, :])
                al_ps = psA.tile([128, 1], F32, tag="alps", bufs=1)
                nc.tensor.matmul(al_ps[:, :], onesr_sb[:, :], alph[:, :],
                                 start=True, stop=True)
                alpha_sb = wpool.tile([128, 1], F32, tag="alphab")
                nc.vector.tensor_copy(alpha_sb[:, :], al_ps[:, :])

                Xb = [nsp.tile([128, 256], F32R, tag=f"X{i}", name=f"X{i}") for i in range(2)]
                XTb = [nsp.tile([128, 256], F32R, tag=f"XT{i}", name=f"XT{i}") for i in range(2)]
                for i in range(2):
                    Mfi = Mf0 if i == 0 else Mf1
                    nc.vector.tensor_scalar_mul(Xb[i][:, :],
                                                Mfi[:, :], alpha_sb[:, :])
                    nc.vector.tensor_copy(XTb[i][:, :], Xb[i][:, :])

            # ================= Phase D: Newton-Schulz =================
            Mr = [Mr0, Mr1]
            Mfp = [Mf0, Mf1]
            with tc.tile_pool(name="psD", bufs=1, space="PSUM") as psD:
                for rnd, (a_coef, dt) in enumerate(NS_ROUNDS):
                    Mop = Mr if dt == F32R else Mfp
                    T_ps = [psD.tile([128, 256], F32, tag=f"T{i}",
                                     name=f"T{i}_{rnd}") for i in range(2)]
                    for i in range(2):
                        nc.tensor.matmul(T_ps[i][:, :],
                                         Mop[0][:, i * 128:(i + 1) * 128],
                                         cast(Xb[0][:, :], dt),
                                         start=True, stop=False)
                        nc.tensor.matmul(T_ps[i][:, :],
                                         Mop[1][:, i * 128:(i + 1) * 128],
                                         cast(Xb[1][:, :], dt),
                                         start=False, stop=True)
                    Ipad = IA_sb if a_coef == 3.5 else I2_sb
                    Wb = [nsp.tile([128, 256], dt, tag=f"W{i}{dt.name}",
                                   name=f"W{i}_{rnd}") for i in range(2)]
                    nc.vector.scalar_tensor_tensor(
                        Wb[0][:, :], T_ps[0][:, :],
                        float(-(a_coef - 1.0)), Ipad[:, 128:384],
                        ALU.mult, ALU.add)
                    nc.vector.scalar_tensor_tensor(
                        Wb[1][:, :], T_ps[1][:, :],
                        float(-(a_coef - 1.0)), Ipad[:, 0:256],
                        ALU.mult, ALU.add)
                    Xn_ps = [psD.tile([128, 256], F32, tag=f"Xn{i}",
                                      name=f"Xn{i}_{rnd}") for i in range(2)]
                    XTn_ps = [psD.tile([128, 256], F32, tag=f"XTn{i}",
                                       name=f"XTn{i}_{rnd}") for i in range(2)]
                    for i in range(2):
                        nc.tensor.matmul(Xn_ps[i][:, :],
                                         cast(XTb[0][:, i * 128:(i + 1) * 128], dt),
                                         Wb[0][:, :], start=True, stop=False)
                        nc.tensor.matmul(Xn_ps[i][:, :],
                                         cast(XTb[1][:, i * 128:(i + 1) * 128], dt),
                                         Wb[1][:, :], start=False, stop=True)
                        nc.tensor.matmul(XTn_ps[i][:, :],
                                         Wb[0][:, i * 128:(i + 1) * 128],
                                         cast(XTb[0][:, :], dt),
                                         start=True, stop=False)
                        nc.tensor.matmul(XTn_ps[i][:, :],
                                         Wb[1][:, i * 128:(i + 1) * 128],
                                         cast(XTb[1][:, :], dt),
                                         start=False, stop=True)
                    ndt = NS_ROUNDS[rnd + 1][1] if rnd + 1 < len(NS_ROUNDS) else F32
                    Xb = [nsp.tile([128, 256], ndt, tag=f"X{i}{ndt.name}",
                                   name=f"X{i}_{rnd}") for i in range(2)]
                    XTb = [nsp.tile([128, 256], ndt, tag=f"XT{i}{ndt.name}",
                                    name=f"XT{i}_{rnd}") for i in range(2)]
                    for i in range(2):
                        nc.vector.tensor_copy(Xb[i][:, :], Xn_ps[i][:, :])
                        nc.vector.tensor_copy(XTb[i][:, :], XTn_ps[i][:, :])

            # ================= Phase E: solve, refine, outputs ==============
            ZT = XTb  # fp32 tiles
            with tc.tile_pool(name="psE", bufs=4, space="PSUM") as psE:
                xb = [wpool.tile([128, 2], F32, tag=f"xb{i}0", name=f"xb{i}_0") for i in range(2)]
                for i in range(2):
                    ps = psE.tile([128, 2], F32, tag="es")
                    nc.tensor.matmul(ps[:, :], ZT[0][:, i * 128:(i + 1) * 128],
                                     bc_sb[:, 0:2], start=True, stop=False)
                    nc.tensor.matmul(ps[:, :], ZT[1][:, i * 128:(i + 1) * 128],
                                     bc_sb[:, 2:4], start=False, stop=True)
                    nc.vector.tensor_copy(xb[i][:, :], ps[:, :])

                for it in range(N_REFINE):
                    r_sb = [wpool.tile([128, 2], F32, tag=f"rsb{i}",
                                       name=f"rsb{i}_{it}") for i in range(2)]
                    for i in range(2):
                        ps = psE.tile([128, 2], F32, tag="es")
                        nc.tensor.matmul(ps[:, :],
                                         Mfp[0][:, i * 128:(i + 1) * 128],
                                         xb[0][:, :], start=True, stop=False)
                        nc.tensor.matmul(ps[:, :],
                                         Mfp[1][:, i * 128:(i + 1) * 128],
                                         xb[1][:, :], start=False, stop=True)
                        nc.vector.scalar_tensor_tensor(
                            r_sb[i][:, :], ps[:, :], -1.0,
                            bc_sb[:, 2 * i:2 * i + 2], ALU.mult, ALU.add)
                    xb_new = [wpool.tile([128, 2], F32, tag=f"xb{i}{it + 1}",
                                         name=f"xb{i}_{it + 1}") for i in range(2)]
                    for i in range(2):
                        ps = psE.tile([128, 2], F32, tag="es")
                        nc.tensor.matmul(ps[:, :],
                                         ZT[0][:, i * 128:(i + 1) * 128],
                                         r_sb[0][:, :], start=True, stop=False)
                        nc.tensor.matmul(ps[:, :],
                                         ZT[1][:, i * 128:(i + 1) * 128],
                                         r_sb[1][:, :], start=False, stop=True)
                        nc.vector.scalar_tensor_tensor(
                            xb_new[i][:, :], ps[:, :], 1.0, xb[i][:, :],
                            ALU.mult, ALU.add)
                    xb = xb_new

                sum_ps = psE.tile([1, 2], F32, tag="es")
                nc.tensor.matmul(sum_ps[:, :], ones_sb[:, :], xb[0][:, :],
                                 start=True, stop=False)
                nc.tensor.matmul(sum_ps[:, :], ones_sb[:, :], xb[1][:, :],
                                 start=False, stop=True)
                sums = wpool.tile([1, 2], F32, tag="sums")
                nc.vector.tensor_copy(sums[:, :], sum_ps[:, :])
                d1 = wpool.tile([1, 1], F32, tag="d1")
                nc.vector.tensor_tensor(d1[:, :], sums[:, 0:1], cst_sb[:, :],
                                        op=ALU.subtract)
                rv = wpool.tile([1, 1], F32, tag="rv")
                nc.vector.reciprocal(rv[:, :], sums[:, 1:2])
                lam = wpool.tile([1, 1], F32, tag="lam")
                nc.vector.tensor_tensor(lam[:, :], d1[:, :], rv[:, :],
                                        op=ALU.mult)
                lam_ps = psE.tile([128, 1], F32, tag="es")
                nc.tensor.matmul(lam_ps[:, :], onesr_sb[:, :], lam[:, :],
                                 start=True, stop=True)
                lamb = wpool.tile([128, 1], F32, tag="lamb")
                nc.vector.tensor_copy(lamb[:, :], lam_ps[:, :])

                qb = [wpool.tile([128, 1], F32, tag=f"qb{i}", name=f"qb{i}") for i in range(2)]
                for i in range(2):
                    nq = wpool.tile([128, 1], F32, tag=f"nq{i}")
                    nc.vector.scalar_tensor_tensor(nq[:, :], xb[i][:, 1:2],
                                                   lamb[:, :], xb[i][:, 0:1],
                                                   ALU.mult, ALU.subtract)
                    nc.vector.tensor_scalar_mul(qb[i][:, :], nq[:, :], -1.0)

                en = [wpool.tile([128, 1], F32, tag=f"en{i}", name=f"en{i}") for i in range(2)]
                for i in range(2):
                    ps = psE.tile([128, 1], F32, tag="es")
                    nc.tensor.matmul(ps[:, :], Mfp[0][:, i * 128:(i + 1) * 128],
                                     qb[0][:, :], start=True, stop=False)
                    nc.tensor.matmul(ps[:, :], Mfp[1][:, i * 128:(i + 1) * 128],
                                     qb[1][:, :], start=False, stop=True)
                    qmq = wpool.tile([128, 1], F32, tag=f"qmq{i}")
                    nc.vector.tensor_tensor(qmq[:, :], qb[i][:, :], ps[:, :],
                                            op=ALU.mult)
                    q2 = wpool.tile([128, 1], F32, tag=f"q2{i}")
                    nc.vector.tensor_tensor(q2[:, :], qb[i][:, :], qb[i][:, :],
                                            op=ALU.mult)
                    jq2 = wpool.tile([128, 1], F32, tag=f"jq2{i}")
                    nc.vector.tensor_tensor(jq2[:, :], q2[:, :],
                                            Jq_sb[:, i:i + 1], op=ALU.mult)
                    nc.vector.tensor_tensor(en[i][:, :], qmq[:, :], jq2[:, :],
                                            op=ALU.subtract)
                e_ps = psE.tile([1, 1], F32, tag="es")
                nc.tensor.matmul(e_ps[:, :], ones_sb[:, :], en[0][:, :],
                                 start=True, stop=False)
                nc.tensor.matmul(e_ps[:, :], ones_sb[:, :], en[1][:, :],
                                 start=False, stop=True)
                e_sb = wpool.tile([1, 1], F32, tag="esb")
                nc.vector.tensor_scalar_mul(e_sb[:, :], e_ps[:, :], 0.5)

                for i in range(2):
                    nc.gpsimd.dma_start(
                        out=qout.ap()[i * 128:(i + 1) * 128, :],
                        in_=qb[i][:, :])
                nc.gpsimd.dma_start(out=eout.ap()[:, :], in_=e_sb[:, :])

    nc.finalize()
    return nc


def _prepare_inputs(inputs):
    """Host-side prep: shard per molecule, build constant tensors."""
    positions = np.asarray(inputs["positions"], np.float32)
    cell = np.asarray(inputs["cell"], np.float32)
    chi = np.asarray(inputs["chi"], np.float32)
    J_raw = np.asarray(inputs["J_raw"], np.float32)
    system_charge = np.asarray(inputs["system_charge"], np.float32)
    atomic_numbers = np.asarray(inputs["atomic_numbers"]).astype(np.int64)

    zmap = np.full(int(ELEMENTS.max()) + 1, -1, np.int64)
    for i, z in enumerate(ELEMENTS):
        zmap[z] = i
    J_elem = np.square(J_raw.astype(np.float64))
    J_i = J_elem[zmap[atomic_numbers]].reshape(B, N)

    Gh = _half_k_grid()                      # [Kh,3] float64 integer grid
    r_all = positions.reshape(B, N, 3)
    chi_b = chi.reshape(B, N)

    in_maps = []
    for m in range(B):
        cm = cell[m].astype(np.float64)
        recip = TWO_PI * np.linalg.inv(cm).T
        kv = Gh @ recip
        k2 = (kv * kv).sum(-1)
        valid = (k2 > 1e-10) & (k2 < (TWO_PI / DL) ** 2)
        Gv = Gh[valid]
        k2v = k2[valid]
        assert len(Gv) <= KPAD, f"valid k count {len(Gv)} exceeds KPAD={KPAD}"
        vol = abs(np.linalg.det(cm))
        scaleA = 2.0 * TWO_PI / vol
        f = np.exp(-0.5 * SIGMA * SIGMA * k2v) / k2v
        w = np.zeros(KPAD, np.float64)
        w[:len(Gv)] = np.sqrt(2.0 * scaleA * f)
        Gp = np.zeros((KPAD, 3), np.float64)
        Gp[:len(Gv)] = Gv

        # Device computes t = g . (r/L-equivalent): use fractional coords
        # frac = r @ inv(cell) so that k.r = 2*pi * g . frac  (exact identity)
        frac = (r_all[m].astype(np.float64) @ np.linalg.inv(cm)).astype(np.float32)
        h = _bf16(frac)
        r1 = (frac - h.astype(np.float32)).astype(np.float32)
        mdl = _bf16(r1)
        lo = _bf16((r1 - mdl.astype(np.float32)).astype(np.float32))

        kmat = np.zeros((10, KPAD), np.float32)
        kmat[0:3] = Gp.T
        kmat[3:6] = Gp.T
        kmat[6:9] = Gp.T
        kmat[9] = 1.0
        rK = np.zeros((10, 512), np.float32)
        for half, shift in ((0, 0.0), (1, 0.25)):
            c0 = half * 256
            rK[0:3, c0:c0 + 256] = h.astype(np.float32).T
            rK[3:6, c0:c0 + 256] = mdl.astype(np.float32).T
            rK[6:9, c0:c0 + 256] = lo.astype(np.float32).T
            rK[9, c0:c0 + 256] = shift

        wcol = w.reshape(KBLK, 128).T.astype(np.float32)   # [128, KBLK]

        dvals = (J_i[m] - GAMMA).astype(np.float32)
        D0 = np.zeros((128, 256), np.float32)
        D0[np.arange(128), np.arange(128)] = dvals[:128]
        D1 = np.zeros((128, 128), np.float32)
        D1[np.arange(128), np.arange(128)] = dvals[128:]

        Ipad = np.zeros((128, 384), np.float32)
        Ipad[np.arange(128), 128 + np.arange(128)] = 1.0

        bc = np.zeros((128, 4), np.float32)
        bc[:, 0] = -chi_b[m, :128]
        bc[:, 1] = 1.0
        bc[:, 2] = -chi_b[m, 128:]
        bc[:, 3] = 1.0

        Jq = np.stack([J_i[m, :128], J_i[m, 128:]], 1).astype(np.float32)

        eye128 = np.eye(128, dtype=np.float32)
        in_maps.append(dict(
            kmat=np.asarray(_bf16(kmat)),
            rK=np.asarray(_bf16(rK)),
            negI=np.asarray(_bf16(-eye128)),
            posI=eye128,
            wcol=wcol,
            D0=D0, D1=D1,
            IA=(3.5 * Ipad).astype(np.float32),
            I2=(2.0 * Ipad).astype(np.float32),
            bc=bc, Jq=Jq,
            cst=np.array([[system_charge[m] / NORM_FACTOR]], np.float32),
            ones=np.ones((128, 1), np.float32),
            onesr=np.ones((1, 128), np.float32),
        ))
    return in_maps


def kernel(**inputs):
    if "nc" not in _CACHE:
        _CACHE["nc"] = _build_program()
    nc = _CACHE["nc"]
    in_maps = _prepare_inputs(inputs)
    res = run_bass_kernel_spmd(nc, in_maps, core_ids=list(range(B)))
    q = np.concatenate([res.results[m]["qout"] for m in range(B)], 0)
    e = np.array([res.results[m]["eout"][0, 0] for m in range(B)], np.float32)
    return q.astype(np.float32), e


# revision 11
# speedup vs baseline: 3924.9931x; 3924.9931x over previous
"""Trainium2 Bass kernel for nn_ChargeEq: per-molecule Ewald matrix build +
constrained charge-equilibration solve, data-parallel over 8 NeuronCores.

Each core handles one molecule:
  - builds the reciprocal-space Ewald matrix A [256,256] on-device:
    half k-space with weights folded in, exact bf16-triple-split phase
    matmul, fp32-magic range reduction, ACT sin, and an exact
    f32r H+R split Gram accumulation (A = HtH + HtR + (HtR)^T).
  - inverts M = A + diag(J) via quadratic Newton-Schulz iteration in
    float32r with an fp32 hybrid tail, then iterative refinement and the
    total-charge constraint.
  - computes q and the Ewald energy e.
"""
import numpy as np

import concourse.bass as bass
import concourse.mybir as mybir
from concourse import bacc
from concourse.tile import TileContext
from concourse.bass_utils import run_bass_kernel_spmd

F32 = mybir.dt.float32
F32R = mybir.dt.float32r
BF16 = mybir.dt.bfloat16
AFT = mybir.ActivationFunctionType
ALU = mybir.AluOpType

# ---- nn.Module constants ----
B, N = 8, 256
DL, SIGMA = 1.5, 1.0
ELEMENTS = np.array([1, 6, 7, 8])
NORM_FACTOR = (1.0 / 90.0474) ** 0.5
TWO_PI = 2.0 * np.pi
NMAX = 10
GAMMA = 2.0 / (SIGMA * np.sqrt(TWO_PI))

KPAD = 2176            # padded half-k-space count (17 * 128)
KBLK = KPAD // 128
MAGIC = 12582912.0     # 1.5 * 2**23 : fp32 round-to-nearest-int trick

ALPHA_C = 2.5          # spectral margin scaling for X0 (max safe ~3.6)
N_GROWTH = 15          # a=3.5 f32r rounds
N_REFINE = 3

_CACHE = {}


def _half_k_grid():
    g = np.arange(-NMAX, NMAX + 1)
    G = np.stack(np.meshgrid(g, g, g, indexing="ij"), -1).reshape(-1, 3)
    G = G.astype(np.float64)
    keep = (G[:, 0] > 0) | ((G[:, 0] == 0) & (G[:, 1] > 0)) | \
           ((G[:, 0] == 0) & (G[:, 1] == 0) & (G[:, 2] > 0))
    return G[keep]


def _bf16(x):
    import ml_dtypes
    return np.asarray(x, np.float32).astype(ml_dtypes.bfloat16)


def _build_program(repeat=1):
    nc = bacc.Bacc(target_bir_lowering=False)

    di = {}
    def dram_in(name, shape, dtype):
        di[name] = nc.dram_tensor(name, shape, dtype, kind="ExternalInput")

    dram_in("kmat", [10, KPAD], BF16)
    dram_in("rK", [10, 512], BF16)
    dram_in("negI", [128, 128], BF16)
    dram_in("posI", [128, 128], F32)
    dram_in("wcol", [128, KBLK], F32)
    dram_in("Dd", [128, 512], F32)
    dram_in("IA", [128, 512], F32)
    dram_in("I2", [128, 512], F32)
    dram_in("bc", [128, 4], F32)
    dram_in("Jq", [128, 2], F32)
    dram_in("cst", [1, 1], F32)
    dram_in("ones", [128, 1], F32)
    dram_in("onesr", [1, 128], F32)

    qout = nc.dram_tensor("qout", [256, 1], F32, kind="ExternalOutput")
    eout = nc.dram_tensor("eout", [1, 1], F32, kind="ExternalOutput")

    def cast(ap, dt):
        return ap if ap.dtype == dt else ap.bitcast(dt)

    with TileContext(nc) as tc:
        with tc.tile_pool(name="cpool", bufs=1) as cpool, \
             tc.tile_pool(name="wpool", bufs=2) as wpool, \
             tc.tile_pool(name="hpool", bufs=4) as hpool, \
             tc.tile_pool(name="mpool", bufs=1) as mpool, \
             tc.tile_pool(name="nsp", bufs=2) as nsp:

            def load(name, shape, dtype):
                t = cpool.tile(shape, dtype, name=f"sb_{name}")
                nc.gpsimd.dma_start(
                    out=t[tuple(slice(0, s) for s in shape)],
                    in_=di[name].ap())
                return t

            kmat_sb = load("kmat", [10, KPAD], BF16)
            rK_sb = load("rK", [10, 512], BF16)
            negI_sb = load("negI", [128, 128], BF16)
            posI_sb = load("posI", [128, 128], F32)
            wcol_sb = load("wcol", [128, KBLK], F32)
            Dd_sb = load("Dd", [128, 512], F32)
            IA_sb = load("IA", [128, 512], F32)
            I2_sb = load("I2", [128, 512], F32)
            bc_sb = load("bc", [128, 4], F32)
            Jq_sb = load("Jq", [128, 2], F32)
            cst_sb = load("cst", [1, 1], F32)
            ones_sb = load("ones", [128, 1], F32)
            onesr_sb = load("onesr", [1, 128], F32)

            for rep in range(repeat):
                # ============ Phase A: build A, assemble M ============
                with tc.tile_pool(name="psA", bufs=2, space="PSUM") as psA, \
                     tc.tile_pool(name="psAcc", bufs=1, space="PSUM") as psAcc:
                    A0_ps = psAcc.tile([128, 256], F32, name=f"A0_{rep}")
                    A1_ps = psAcc.tile([128, 256], F32, name=f"A1_{rep}")
                    U0_ps = psAcc.tile([128, 256], F32, name=f"U0_{rep}")
                    U1_ps = psAcc.tile([128, 256], F32, name=f"U1_{rep}")
                    for b in range(KBLK):
                        c0 = b * 128
                        t_ps = psA.tile([128, 512], F32, tag="tps", bufs=3,
                                        name=f"tps_{rep}_{b}")
                        nc.tensor.matmul(t_ps[:, :], kmat_sb[:, c0:c0 + 128],
                                         rK_sb[:, :], start=True, stop=False)
                        a1 = hpool.tile([128, 512], F32, tag="a1",
                                        name=f"a1_{rep}_{b}")
                        nc.scalar.activation(a1[:, :], t_ps[:, :], AFT.Copy,
                                             bias=MAGIC)
                        rintb = hpool.tile([128, 512], BF16, tag="rintb",
                                           name=f"rintb_{rep}_{b}")
                        nc.vector.tensor_scalar_sub(rintb[:, :], a1[:, :],
                                                    MAGIC)
                        nc.tensor.matmul(t_ps[:, :], negI_sb[:, :],
                                         rintb[:, :], start=False, stop=True)
                        snc = hpool.tile([128, 512], F32, tag="snc",
                                         name=f"snc_{rep}_{b}")
                        nc.scalar.activation(snc[:, :], t_ps[:, :], AFT.Sin,
                                             scale=float(TWO_PI))
                        H = hpool.tile([128, 512], F32R, tag="H",
                                       name=f"H_{rep}_{b}")
                        nc.vector.tensor_scalar_mul(H[:, :], snc[:, :],
                                                    wcol_sb[:, b:b + 1])
                        R = hpool.tile([128, 512], F32R, tag="R",
                                       name=f"R_{rep}_{b}")
                        nc.vector.scalar_tensor_tensor(
                            R[:, :], snc[:, :], wcol_sb[:, b:b + 1],
                            cast(H[:, :], F32), ALU.mult, ALU.subtract)
                        st = (b == 0)
                        sp = (b == KBLK - 1)
                        nc.tensor.matmul(A0_ps[:, :], H[:, 0:128],
                                         H[:, 0:256], start=st, stop=False)
                        nc.tensor.matmul(A0_ps[:, :], H[:, 256:384],
                                         H[:, 256:512], start=False,
                                         stop=False)
                        nc.tensor.matmul(A1_ps[:, :], H[:, 128:256],
                                         H[:, 0:256], start=st, stop=False)
                        nc.tensor.matmul(A1_ps[:, :], H[:, 384:512],
                                         H[:, 256:512], start=False,
                                         stop=False)
                        nc.tensor.matmul(U0_ps[:, :], H[:, 0:128],
                                         R[:, 0:256], start=st, stop=False)
                        nc.tensor.matmul(U0_ps[:, :], H[:, 256:384],
                                         R[:, 256:512], start=False, stop=sp)
                        nc.tensor.matmul(U1_ps[:, :], H[:, 128:256],
                                         R[:, 0:256], start=st, stop=False)
                        nc.tensor.matmul(U1_ps[:, :], H[:, 384:512],
                                         R[:, 256:512], start=False, stop=sp)

                    Ucp0 = wpool.tile([128, 256], F32, name=f"Ucp0_{rep}")
                    nc.vector.tensor_copy(Ucp0[:, :], U0_ps[:, :])
                    Ucp1 = wpool.tile([128, 256], F32, name=f"Ucp1_{rep}")
                    nc.scalar.copy(Ucp1[:, :], U1_ps[:, :])
                    nc.tensor.matmul(A0_ps[:, 0:128], Ucp0[:, 0:128],
                                     posI_sb[:, :], is_transpose=True,
                                     start=False, stop=False)
                    nc.tensor.matmul(A0_ps[:, 128:256], Ucp1[:, 0:128],
                                     posI_sb[:, :], is_transpose=True,
                                     start=False, stop=True)
                    nc.tensor.matmul(A1_ps[:, 0:128], Ucp0[:, 128:256],
                                     posI_sb[:, :], is_transpose=True,
                                     start=False, stop=False)
                    nc.tensor.matmul(A1_ps[:, 128:256], Ucp1[:, 128:256],
                                     posI_sb[:, :], is_transpose=True,
                                     start=False, stop=True)
                    UD0 = wpool.tile([128, 256], F32, name=f"UD0_{rep}")
                    nc.vector.tensor_tensor(UD0[:, :], Ucp0[:, :],
                                            Dd_sb[:, 0:256], op=ALU.add)
                    UD1 = wpool.tile([128, 256], F32, name=f"UD1_{rep}")
                    nc.vector.tensor_tensor(UD1[:, :], Ucp1[:, :],
                                            Dd_sb[:, 256:512], op=ALU.add)
                    Mf = mpool.tile([128, 512], F32, name=f"Mf_{rep}")
                    nc.vector.scalar_tensor_tensor(Mf[:, 0:256], A0_ps[:, :],
                                                   1.0, UD0[:, :], ALU.mult,
                                                   ALU.add)
                    nc.vector.scalar_tensor_tensor(Mf[:, 256:512],
                                                   A1_ps[:, :], 1.0,
                                                   UD1[:, :], ALU.mult,
                                                   ALU.add)
                    Mr = mpool.tile([128, 512], F32R, name=f"Mr_{rep}")
                    nc.vector.tensor_copy(Mr[:, :], Mf[:, :])

                    rs0 = wpool.tile([128, 1], F32, tag="rs0",
                                     name=f"rs0_{rep}")
                    nc.vector.tensor_reduce(rs0[:, :], Mf[:, 0:256],
                                            axis=mybir.AxisListType.X,
                                            op=ALU.add,
                                            apply_absolute_value=True)
                    rs1 = wpool.tile([128, 1], F32, tag="rs1",
                                     name=f"rs1_{rep}")
                    nc.vector.tensor_reduce(rs1[:, :], Mf[:, 256:512],
                                            axis=mybir.AxisListType.X,
                                            op=ALU.add,
                                            apply_absolute_value=True)
                    rsmax = wpool.tile([128, 1], F32, tag="rsmax",
                                       name=f"rsmax_{rep}")
                    nc.vector.tensor_tensor(rsmax[:, :], rs0[:, :],
                                            rs1[:, :], op=ALU.max)
                    rst_ps = psA.tile([1, 128], F32, tag="smallA", bufs=1,
                                      name=f"rst_{rep}")
                    nc.tensor.transpose(rst_ps[:, :], rsmax[:, :],
                                        posI_sb[:, :])
                    nrm = wpool.tile([1, 1], F32, tag="nrm",
                                     name=f"nrm_{rep}")
                    nc.vector.tensor_reduce(nrm[:, :], rst_ps[:, :],
                                            axis=mybir.AxisListType.X,
                                            op=ALU.max)
                    nrm2 = wpool.tile([1, 1], F32, tag="nrm2",
                                      name=f"nrm2_{rep}")
                    nc.vector.tensor_tensor(nrm2[:, :], nrm[:, :],
                                            nrm[:, :], op=ALU.mult)
                    alph = wpool.tile([1, 1], F32, tag="alph",
                                      name=f"alph_{rep}")
                    nc.vector.reciprocal(alph[:, :], nrm2[:, :])
                    alc = wpool.tile([1, 1], F32, tag="alc",
                                     name=f"alc_{rep}")
                    nc.vector.tensor_scalar_mul(alc[:, :], alph[:, :],
                                                float(ALPHA_C))
                    al_ps = psA.tile([128, 1], F32, tag="smallA", bufs=1,
                                     name=f"alps_{rep}")
                    nc.tensor.matmul(al_ps[:, :], onesr_sb[:, :],
                                     alc[:, :], start=True, stop=True)
                    alpha_sb = wpool.tile([128, 1], F32, tag="alphab",
                                          name=f"alphab_{rep}")
                    nc.vector.tensor_copy(alpha_sb[:, :], al_ps[:, :])

                    X = nsp.tile([128, 512], F32R, tag="Xr",
                                 name=f"X0_{rep}")
                    XT = nsp.tile([128, 512], F32R, tag="XTr",
                                  name=f"XT0_{rep}")
                    nc.vector.tensor_scalar_mul(X[:, :], Mf[:, :],
                                                alpha_sb[:, :])
                    nc.vector.tensor_copy(XT[:, :], X[:, :])

                # ============ Phase D: Newton-Schulz ============
                rounds = [(3.5, "r")] * N_GROWTH + [(2.0, "r")] * 2 + \
                         [(2.0, "f")]
                with tc.tile_pool(name="psD", bufs=2, space="PSUM") as psD:
                    for rnd, (a_coef, mode) in enumerate(rounds):
                        rdt = F32R if mode == "r" else F32
                        Mop = Mr
                        T_ps = psD.tile([128, 512], F32, tag="T",
                                        name=f"T_{rep}_{rnd}")
                        for i in range(2):
                            nc.tensor.matmul(
                                T_ps[:, 256 * i:256 * i + 256],
                                Mop[:, 128 * i:128 * i + 128],
                                cast(X[:, 0:256], F32R),
                                start=True, stop=False)
                            nc.tensor.matmul(
                                T_ps[:, 256 * i:256 * i + 256],
                                Mop[:, 256 + 128 * i:256 + 128 * i + 128],
                                cast(X[:, 256:512], F32R),
                                start=False, stop=True)
                        Ipad = IA_sb if a_coef == 3.5 else I2_sb
                        Wb = [nsp.tile([128, 256], rdt,
                                       tag=f"W{i}{rdt.name}",
                                       name=f"W{i}_{rep}_{rnd}")
                              for i in range(2)]
                        for i in range(2):
                            nc.vector.scalar_tensor_tensor(
                                Wb[i][:, :],
                                T_ps[:, 256 * i:256 * i + 256],
                                float(-(a_coef - 1.0)),
                                Ipad[:, 256 * i:256 * i + 256],
                                ALU.mult, ALU.add)
                        if mode != "f":
                            Xn_ps = psD.tile([128, 512], F32, tag="Xn",
                                             name=f"Xn_{rep}_{rnd}")
                            for i in range(2):
                                nc.tensor.matmul(
                                    Xn_ps[:, 256 * i:256 * i + 256],
                                    cast(XT[:, 128 * i:128 * i + 128], rdt),
                                    Wb[0][:, :], start=True, stop=False)
                                nc.tensor.matmul(
                                    Xn_ps[:, 256 * i:256 * i + 256],
                                    cast(XT[:, 256 + 128 * i:
                                            256 + 128 * i + 128], rdt),
                                    Wb[1][:, :], start=False, stop=True)
                        XTn_ps = psD.tile([128, 512], F32, tag="XTn",
                                          name=f"XTn_{rep}_{rnd}")
                        for i in range(2):
                            nc.tensor.matmul(
                                XTn_ps[:, 256 * i:256 * i + 256],
                                Wb[0][:, 128 * i:128 * i + 128],
                                cast(XT[:, 0:256], rdt),
                                start=True, stop=False)
                            nc.tensor.matmul(
                                XTn_ps[:, 256 * i:256 * i + 256],
                                Wb[1][:, 128 * i:128 * i + 128],
                                cast(XT[:, 256:512], rdt),
                                start=False, stop=True)
                        if mode == "f":
                            XT = nsp.tile([128, 512], F32, tag="XTfin",
                                          name=f"XT_{rep}_{rnd}")
                            for i in range(2):
                                nc.vector.tensor_copy(
                                    XT[:, 256 * i:256 * i + 256],
                                    XTn_ps[:, 256 * i:256 * i + 256])
                            continue
                        X = nsp.tile([128, 512], F32R, tag="Xr2",
                                     name=f"X_{rep}_{rnd}")
                        XT = nsp.tile([128, 512], F32R, tag="XTr2",
                                      name=f"XT_{rep}_{rnd}")
                        for i in range(2):
                            nc.vector.tensor_copy(
                                X[:, 256 * i:256 * i + 256],
                                Xn_ps[:, 256 * i:256 * i + 256])
                            nc.scalar.copy(
                                XT[:, 256 * i:256 * i + 256],
                                XTn_ps[:, 256 * i:256 * i + 256])

                # ============ Phase E: solve, refine, outputs ============
                ZT = XT
                with tc.tile_pool(name="psE", bufs=4, space="PSUM") as psE:
                    def apply_op(lhs, rhs_tile, tag):
                        ps = psE.tile([128, 4], F32, tag="es", name=tag)
                        for i in range(2):
                            nc.tensor.matmul(
                                ps[:, 2 * i:2 * i + 2],
                                lhs[:, 128 * i:128 * i + 128],
                                rhs_tile[:, 0:2], start=True, stop=False)
                            nc.tensor.matmul(
                                ps[:, 2 * i:2 * i + 2],
                                lhs[:, 256 + 128 * i:256 + 128 * i + 128],
                                rhs_tile[:, 2:4], start=False, stop=True)
                        return ps

                    x_ps = apply_op(ZT, bc_sb, f"x0_{rep}")
                    xb = wpool.tile([128, 4], F32, tag="xb",
                                    name=f"xb_{rep}")
                    nc.vector.tensor_copy(xb[:, :], x_ps[:, :])

                    for it in range(N_REFINE):
                        r_ps = apply_op(Mf, xb, f"rp_{rep}_{it}")
                        r_sb = wpool.tile([128, 4], F32, tag="rsb",
                                          name=f"rsb_{rep}_{it}")
                        nc.vector.scalar_tensor_tensor(
                            r_sb[:, :], r_ps[:, :], -1.0, bc_sb[:, :],
                            ALU.mult, ALU.add)
                        dx_ps = apply_op(ZT, r_sb, f"dx_{rep}_{it}")
                        xb_new = wpool.tile([128, 4], F32, tag="xb",
                                            name=f"xb_{rep}_{it + 1}")
                        nc.vector.scalar_tensor_tensor(
                            xb_new[:, :], dx_ps[:, :], 1.0, xb[:, :],
                            ALU.mult, ALU.add)
                        xb = xb_new

                    sum_ps = psE.tile([1, 4], F32, tag="es",
                                      name=f"sums_{rep}")
                    nc.tensor.matmul(sum_ps[:, :], ones_sb[:, :], xb[:, :],
                                     start=True, stop=True)
                    sums = wpool.tile([1, 4], F32, tag="sums",
                                      name=f"sumsb_{rep}")
                    nc.vector.tensor_copy(sums[:, :], sum_ps[:, :])
                    suv = wpool.tile([1, 2], F32, tag="suv",
                                     name=f"suv_{rep}")
                    nc.vector.tensor_tensor(suv[:, :], sums[:, 0:2],
                                            sums[:, 2:4], op=ALU.add)
                    d1 = wpool.tile([1, 1], F32, tag="d1", name=f"d1_{rep}")
                    nc.vector.tensor_tensor(d1[:, :], suv[:, 0:1],
                                            cst_sb[:, :], op=ALU.subtract)
                    rv = wpool.tile([1, 1], F32, tag="rv", name=f"rv_{rep}")
                    nc.vector.reciprocal(rv[:, :], suv[:, 1:2])
                    lam = wpool.tile([1, 1], F32, tag="lam",
                                     name=f"lam_{rep}")
                    nc.vector.tensor_tensor(lam[:, :], d1[:, :], rv[:, :],
                                            op=ALU.mult)
                    lam_ps = psE.tile([128, 1], F32, tag="es",
                                      name=f"lmps_{rep}")
                    nc.tensor.matmul(lam_ps[:, :], onesr_sb[:, :],
                                     lam[:, :], start=True, stop=True)
                    lamb = wpool.tile([128, 1], F32, tag="lamb",
                                      name=f"lamb_{rep}")
                    nc.vector.tensor_copy(lamb[:, :], lam_ps[:, :])

                    qb = wpool.tile([128, 2], F32, tag="qb",
                                    name=f"qb_{rep}")
                    for i in range(2):
                        nq = wpool.tile([128, 1], F32, tag=f"nq{i}",
                                        name=f"nq{i}_{rep}")
                        nc.vector.scalar_tensor_tensor(
                            nq[:, :], xb[:, 2 * i + 1:2 * i + 2],
                            lamb[:, :], xb[:, 2 * i:2 * i + 1],
                            ALU.mult, ALU.subtract)
                        nc.vector.tensor_scalar_mul(qb[:, i:i + 1],
                                                    nq[:, :], -1.0)

                    mq_ps = psE.tile([128, 2], F32, tag="es",
                                     name=f"mq_{rep}")
                    for i in range(2):
                        nc.tensor.matmul(
                            mq_ps[:, i:i + 1],
                            Mf[:, 128 * i:128 * i + 128],
                            qb[:, 0:1], start=True, stop=False)
                        nc.tensor.matmul(
                            mq_ps[:, i:i + 1],
                            Mf[:, 256 + 128 * i:256 + 128 * i + 128],
                            qb[:, 1:2], start=False, stop=True)
                    qmq = wpool.tile([128, 2], F32, tag="qmq",
                                     name=f"qmq_{rep}")
                    nc.vector.tensor_tensor(qmq[:, :], qb[:, :],
                                            mq_ps[:, :], op=ALU.mult)
                    jq = wpool.tile([128, 2], F32, tag="jq",
                                    name=f"jq_{rep}")
                    nc.vector.tensor_tensor(jq[:, :], qb[:, :],
                                            Jq_sb[:, :], op=ALU.mult)
                    jq2 = wpool.tile([128, 2], F32, tag="jq2",
                                     name=f"jq2_{rep}")
                    nc.vector.tensor_tensor(jq2[:, :], jq[:, :], qb[:, :],
                                            op=ALU.mult)
                    en = wpool.tile([128, 2], F32, tag="en",
                                    name=f"en_{rep}")
                    nc.vector.tensor_tensor(en[:, :], qmq[:, :],
                                            jq2[:, :], op=ALU.subtract)
                    e_ps = psE.tile([1, 2], F32, tag="es",
                                    name=f"eps_{rep}")
                    nc.tensor.matmul(e_ps[:, :], ones_sb[:, :], en[:, :],
                                     start=True, stop=True)
                    ecol = wpool.tile([1, 2], F32, tag="ecol",
                                      name=f"ecol_{rep}")
                    nc.vector.tensor_copy(ecol[:, :], e_ps[:, :])
                    esum = wpool.tile([1, 1], F32, tag="esum",
                                      name=f"esum_{rep}")
                    nc.vector.tensor_tensor(esum[:, :], ecol[:, 0:1],
                                            ecol[:, 1:2], op=ALU.add)
                    e_sb = wpool.tile([1, 1], F32, tag="esb",
                                      name=f"esb_{rep}")
                    nc.vector.tensor_scalar_mul(e_sb[:, :], esum[:, :], 0.5)

                    if rep == repeat - 1:
                        nc.gpsimd.dma_start(
                            out=qout.ap().rearrange(
                                "(c p) one -> p (c one)", p=128),
                            in_=qb[:, :])
                        nc.gpsimd.dma_start(out=eout.ap()[:, :],
                                            in_=e_sb[:, :])

    nc.finalize()
    return nc


def _prepare_inputs(inputs):
    positions = np.asarray(inputs["positions"], np.float32)
    cell = np.asarray(inputs["cell"], np.float32)
    chi = np.asarray(inputs["chi"], np.float32)
    J_raw = np.asarray(inputs["J_raw"], np.float32)
    system_charge = np.asarray(inputs["system_charge"], np.float32)
    atomic_numbers = np.asarray(inputs["atomic_numbers"]).astype(np.int64)

    zmap = np.full(int(ELEMENTS.max()) + 1, -1, np.int64)
    for i, z in enumerate(ELEMENTS):
        zmap[z] = i
    J_elem = np.square(J_raw.astype(np.float64))
    J_i = J_elem[zmap[atomic_numbers]].reshape(B, N)

    Gh = _half_k_grid()
    r_all = positions.reshape(B, N, 3)
    chi_b = chi.reshape(B, N)

    in_maps = []
    for m in range(B):
        cm = cell[m].astype(np.float64)
        recip = TWO_PI * np.linalg.inv(cm).T
        kv = Gh @ recip
        k2 = (kv * kv).sum(-1)
        valid = (k2 > 1e-10) & (k2 < (TWO_PI / DL) ** 2)
        Gv = Gh[valid]
        k2v = k2[valid]
        assert len(Gv) <= KPAD, f"valid k count {len(Gv)} exceeds KPAD={KPAD}"
        vol = abs(np.linalg.det(cm))
        scaleA = 2.0 * TWO_PI / vol
        f = np.exp(-0.5 * SIGMA * SIGMA * k2v) / k2v
        w = np.zeros(KPAD, np.float64)
        w[:len(Gv)] = np.sqrt(2.0 * scaleA * f)
        Gp = np.zeros((KPAD, 3), np.float64)
        Gp[:len(Gv)] = Gv

        frac = (r_all[m].astype(np.float64) @
                np.linalg.inv(cm)).astype(np.float32)
        h = _bf16(frac)
        r1 = (frac - h.astype(np.float32)).astype(np.float32)
        mdl = _bf16(r1)
        lo = _bf16((r1 - mdl.astype(np.float32)).astype(np.float32))

        kmat = np.zeros((10, KPAD), np.float32)
        kmat[0:3] = Gp.T
        kmat[3:6] = Gp.T
        kmat[6:9] = Gp.T
        kmat[9] = 1.0
        rK = np.zeros((10, 512), np.float32)
        for half, shift in ((0, 0.0), (1, 0.25)):
            c0 = half * 256
            rK[0:3, c0:c0 + 256] = h.astype(np.float32).T
            rK[3:6, c0:c0 + 256] = mdl.astype(np.float32).T
            rK[6:9, c0:c0 + 256] = lo.astype(np.float32).T
            rK[9, c0:c0 + 256] = shift

        wcol = w.reshape(KBLK, 128).T.astype(np.float32)

        dvals = (J_i[m] - GAMMA).astype(np.float32)
        Dd = np.zeros((128, 512), np.float32)
        Dd[np.arange(128), np.arange(128)] = dvals[:128]
        Dd[np.arange(128), 256 + 128 + np.arange(128)] = dvals[128:]

        Ipad = np.zeros((128, 512), np.float32)
        Ipad[np.arange(128), np.arange(128)] = 1.0
        Ipad[np.arange(128), 256 + 128 + np.arange(128)] = 1.0

        bc = np.zeros((128, 4), np.float32)
        bc[:, 0] = -chi_b[m, :128]
        bc[:, 1] = 1.0
        bc[:, 2] = -chi_b[m, 128:]
        bc[:, 3] = 1.0

        Jq = np.stack([J_i[m, :128], J_i[m, 128:]], 1).astype(np.float32)

        eye128 = np.eye(128, dtype=np.float32)
        in_maps.append(dict(
            kmat=np.asarray(_bf16(kmat)),
            rK=np.asarray(_bf16(rK)),
            negI=np.asarray(_bf16(-eye128)),
            posI=eye128,
            wcol=wcol,
            Dd=Dd,
            IA=(3.5 * Ipad).astype(np.float32),
            I2=(2.0 * Ipad).astype(np.float32),
            bc=bc, Jq=Jq,
            cst=np.array([[system_charge[m] / NORM_FACTOR]], np.float32),
            ones=np.ones((128, 1), np.float32),
            onesr=np.ones((1, 128), np.float32),
        ))
    return in_maps


def kernel(**inputs):
    if "nc" not in _CACHE:
        _CACHE["nc"] = _build_program()
    nc = _CACHE["nc"]
    in_maps = _prepare_inputs(inputs)
    res = run_bass_kernel_spmd(nc, in_maps, core_ids=list(range(B)))
    q = np.concatenate([res.results[m]["qout"] for m in range(B)], 0)
    e = np.array([res.results[m]["eout"][0, 0] for m in range(B)], np.float32)
    return q.astype(np.float32), e
